# revision 15
# baseline (speedup 1.0000x reference)
"""Trainium2 kernel for nn_Experts (MoE grouped expert GEMM).

Problem: input [16384, 2048] f32, weight [8, 8192, 2048] f32, bias [8, 8192]
f32, expert_frequency [8] int32 (balanced: 2048 tokens/expert, pre-grouped),
capacity 2048.  Output [16384, 8192] f32 with out[t] = W_e x[t] + b_e.

Sharding: expert parallelism — core e computes expert e's GEMM
  Y_e = X_e @ W_e^T + b_e   (X_e [2048, 2048], W_e [8192, 2048])

Per-core kernel computes YT_e = W_e X_e^T + b_e  ([OUT, TOK], transposed
output; the host transposes back).  Matmul precision: single-pass bf16
(inputs rounded to bf16, fp32 PSUM accumulation) — carries ~2e-3 relative
error on this problem's randn data, comfortably under the 2e-2 gate, at
3x the throughput of the old split-precision bf16x3 scheme.

DMA-descriptor discipline (the previous bottleneck): every DMA moves
contiguous >=2KB lines per partition.  Weights are pre-swizzled on the
host to [OT, 128, KC*128] so one out-tile's weights are a single
128-descriptor (4KB/partition) transfer.  Output is written one out-tile
at a time ([128, TOK] f32, 8KB lines).

Engine layout (raw Bass, single-sem waits):
  SP   : input DMAs (w tile stream, x chunks, bias)
  PE   : 4096 matmuls ([128,128] stationary w x [128,512] moving x),
         k-outer: per (o,k) the stationary is reused for 4 consecutive
         t-slice matmuls; 8 PSUM banks, 4 per out-tile, ping-pong
  DVE  : PSUM -> SBUF eviction fused with per-partition bias add
  ACT  : output DMAs (one per out-tile)

Semaphore-correctness note: a DMA completion count of 16*n on a shared
ring proves only that SOME n DMAs finished, not WHICH — concurrent ring
DMAs complete out of order near stream start.  Tight per-chunk waits on
the x stream corrupted the first out-tiles nondeterministically; the PE
therefore waits for ALL x chunks (and w0+w1) before its first matmul.
All later waits carry >=10us of natural pipeline slack.  Measured:
~0.80 ms/core HW exec (PE streaming roofline ~0.87 ms), rel err 2.0e-3
vs the f32 reference (gate 2e-2).
"""

import numpy as np

import concourse.bass as bass
import concourse.mybir as mybir
from concourse.bass_utils import run_bass_kernel_spmd

# problem shape (per core)
E = 8
TOK = 2048      # tokens per expert (= capacity)
IN = 2048       # in features (contraction)
OUT = 8192      # out features
T_FULL = E * TOK

KC = IN // 128          # 16 contraction chunks
SLICE = 512             # moving-dim (token) slice = one PSUM bank
TS = TOK // SLICE       # 4 token slices
OT = OUT // 128         # 64 out tiles
NW = 4                  # w slot rotation
NP2 = 8                 # psum banks

F32 = mybir.dt.float32

_MODES = {
    # mode: mm dtype (single-pass, fp32 PSUM accumulate)
    "bf16": mybir.dt.bfloat16,
    "fp16": mybir.dt.float16,
}
MODE = "bf16"


def _enable_ldw_opt():
    """Flip walrus --enable-ldw-opt to true (elides identical consecutive
    LDWEIGHTS; the k-outer loop reuses each stationary 4x)."""
    import concourse.bass_utils as bu
    if getattr(bu.run_command, "_ldw_patched", False):
        return
    real_run = bu.run_command

    def run_hook(cmd, **kw):
        try:
            cmd = ["--enable-ldw-opt=true" if c == "--enable-ldw-opt=false" else c
                   for c in cmd]
        except Exception:
            pass
        return real_run(cmd, **kw)

    run_hook._ldw_patched = True
    bu.run_command = run_hook


def build(mode: str = MODE, reps: int = 1, bench: bool = False) -> bass.Bass:
    """reps: run the whole kernel body that many times back-to-back (for
    marginal-time benchmarking).  bench: make yt an internal DRAM scratch
    and expose only a tiny marker output, so per-call host<->device
    transfer is negligible during timing."""
    import os
    mm_dt = _MODES[mode]
    if os.environ.get("LDW_OPT", "0") != "0":
        _enable_ldw_opt()

    nc = bass.Bass(target_bir_lowering=False)
    xt = nc.dram_tensor("xt", [IN, TOK], mm_dt, kind="ExternalInput")
    wt = nc.dram_tensor("wt", [OT, 128, KC * 128], mm_dt, kind="ExternalInput")
    bias = nc.dram_tensor("bias", [128, OT], F32, kind="ExternalInput")
    if bench:
        yt = nc.dram_tensor("yt", [OUT, TOK], F32)  # internal scratch
        marker = nc.dram_tensor("marker", [128, OT], F32, kind="ExternalOutput")
    else:
        assert reps == 1
        yt = nc.dram_tensor("yt", [OUT, TOK], F32, kind="ExternalOutput")

    # [128, KC, TOK]: chunk c covers input rows c*128 .. c*128+127
    xt_r = xt[:, :].rearrange("(c p) t -> p c t", p=128)

    with (
        nc.sbuf_tensor("x_sb", [128, KC, TOK], mm_dt) as x_sb,
        nc.sbuf_tensor("w_sb", [128, NW, KC, 128], mm_dt) as w_sb,
        nc.sbuf_tensor("y_sb", [128, 2, TOK], F32) as y_sb,
        nc.sbuf_tensor("b_sb", [128, OT], F32) as b_sb,
        nc.psum_tensor("acc", [128, NP2, SLICE], F32) as acc,
        nc.semaphore("sem_x") as sem_x,
        nc.semaphore("sem_b") as sem_b,
        nc.semaphore("sem_w") as sem_w,
        nc.semaphore("sem_pe") as sem_pe,
        nc.semaphore("sem_dve") as sem_dve,
        nc.semaphore("sem_dout") as sem_dout,
        nc.Block() as block,
    ):
        WO = reps * OT
        x_done = [16 * (k + 1) for k in range(KC)]   # sem_x after chunk k

        @block.sync
        def _(sp):
            # w tile 0 first (PE's first dependency), then bias + x chunks
            # (PE consumes them k-ascending inside wo=0), then the w stream.
            sp.dma_start(w_sb[:, 0, :, :], wt[0]).then_inc(sem_w, 16)
            sp.dma_start(b_sb[:], bias[:]).then_inc(sem_b, 16)
            for k in range(KC):
                sp.dma_start(x_sb[:, k, :], xt_r[:, k, :]).then_inc(sem_x, 16)
            for wo in range(1, WO):
                o = wo % OT
                if wo >= NW:
                    # PE done reading slot wo-NW once sem_pe >= wo-NW+1
                    sp.wait_ge(sem_pe, wo - NW + 1)
                sp.dma_start(w_sb[:, wo % NW, :, :], wt[o]).then_inc(sem_w, 16)
            sp.wait_ge(sem_dout, 16 * WO)
            if bench:
                sp.dma_start(marker[:, :], b_sb[:]).then_inc(sem_x, 16)

        @block.tensor
        def _(pe):
            for wo in range(WO):
                pe.wait_ge(sem_w, 16 * (wo + 1) if wo else 32)
                if wo == 0:
                    # wait for ALL x chunks (and w0+w1): DMA completion
                    # counts don't identify WHICH dma finished, and ring
                    # completions reorder at stream start.  All later waits
                    # have >=10us of natural slack on top of the counts.
                    pe.wait_ge(sem_x, 16 * KC)
                if wo >= 2:
                    # banks (wo%2)*4.. freed once DVE evicted wo-2's tiles
                    pe.wait_ge(sem_dve, TS * (wo - 1))
                base = (wo % 2) * TS
                for k in range(KC):
                    for t in range(TS):
                        mm = pe.matmul(
                            acc[:, base + t, :],
                            w_sb[:, wo % NW, k, :],
                            x_sb[:, k, t * SLICE:(t + 1) * SLICE],
                            start=(k == 0),
                            stop=(k == KC - 1),
                        )
                mm.then_inc(sem_pe, 1)

        @block.vector
        def _(dve):
            for wo in range(WO):
                o = wo % OT
                dve.wait_ge(sem_pe, wo + 1)
                if wo == 0:
                    dve.wait_ge(sem_b, 16)
                if wo >= 2:
                    # y slot wo%2 free once wo-2's output DMA completed
                    dve.wait_ge(sem_dout, 16 * (wo - 1))
                base = (wo % 2) * TS
                for t in range(TS):
                    dve.tensor_scalar_add(
                        y_sb[:, wo % 2, t * SLICE:(t + 1) * SLICE],
                        acc[:, base + t, :],
                        b_sb[:, o:o + 1],
                    ).then_inc(sem_dve, 1)

        @block.scalar
        def _(act):
            for wo in range(WO):
                o = wo % OT
                act.wait_ge(sem_dve, TS * (wo + 1))
                act.dma_start(
                    yt[o * 128:(o + 1) * 128, :],
                    y_sb[:, wo % 2, :],
                ).then_inc(sem_dout, 16)

    return nc


def build_pe(mode: str = MODE, reps: int = 1, bench: bool = True) -> bass.Bass:
    """Bench-only probe: PE + weight-DMA loop with NO eviction/output path.
    Measures the pure matmul issue rate ceiling."""
    import os
    mm_dt = _MODES[mode]
    if os.environ.get("LDW_OPT", "0") != "0":
        _enable_ldw_opt()
    assert bench

    nc = bass.Bass(target_bir_lowering=False)
    xt = nc.dram_tensor("xt", [IN, TOK], mm_dt, kind="ExternalInput")
    wt = nc.dram_tensor("wt", [OT, 128, KC * 128], mm_dt, kind="ExternalInput")
    bias = nc.dram_tensor("bias", [128, OT], F32, kind="ExternalInput")
    marker = nc.dram_tensor("marker", [128, OT], F32, kind="ExternalOutput")

    xt_r = xt[:, :].rearrange("(c p) t -> p c t", p=128)

    with (
        nc.sbuf_tensor("x_sb", [128, KC, TOK], mm_dt) as x_sb,
        nc.sbuf_tensor("w_sb", [128, NW, KC, 128], mm_dt) as w_sb,
        nc.sbuf_tensor("b_sb", [128, OT], F32) as b_sb,
        nc.psum_tensor("acc", [128, NP2, SLICE], F32) as acc,
        nc.semaphore("sem_x") as sem_x,
        nc.semaphore("sem_b") as sem_b,
        nc.semaphore("sem_w") as sem_w,
        nc.semaphore("sem_pe") as sem_pe,
        nc.Block() as block,
    ):
        WO = reps * OT

        @block.sync
        def _(sp):
            sp.dma_start(w_sb[:, 0, :, :], wt[0]).then_inc(sem_w, 16)
            sp.dma_start(b_sb[:], bias[:]).then_inc(sem_b, 16)
            for k in range(KC):
                sp.dma_start(x_sb[:, k, :], xt_r[:, k, :]).then_inc(sem_x, 16)
            for wo in range(1, WO):
                o = wo % OT
                if wo >= NW:
                    sp.wait_ge(sem_pe, wo - NW + 1)
                sp.dma_start(w_sb[:, wo % NW, :, :], wt[o]).then_inc(sem_w, 16)
            sp.wait_ge(sem_pe, WO)
            sp.dma_start(marker[:, :], b_sb[:]).then_inc(sem_b, 16)

        @block.tensor
        def _(pe):
            for wo in range(WO):
                pe.wait_ge(sem_w, 16 * min(wo + 2, WO))
                if wo == 0:
                    pe.wait_ge(sem_x, 16 * KC)
                base = (wo % 2) * TS
                for k in range(KC):
                    for t in range(TS):
                        mm = pe.matmul(
                            acc[:, base + t, :],
                            w_sb[:, wo % NW, k, :],
                            x_sb[:, k, t * SLICE:(t + 1) * SLICE],
                            start=(k == 0),
                            stop=(k == KC - 1),
                        )
                mm.then_inc(sem_pe, 1)

    return nc


_nc_cache: dict = {}


def _get_nc(mode: str) -> bass.Bass:
    if mode not in _nc_cache:
        _nc_cache[mode] = build(mode)
    return _nc_cache[mode]


def _make_in_maps(input, weight, bias, expert_frequency, mode: str):
    mm_dt = _MODES[mode]
    np_dt = mybir.dt.np(mm_dt)

    freq = np.asarray(expert_frequency, dtype=np.int64)
    ends = np.cumsum(freq)
    starts = ends - freq

    input = np.asarray(input, dtype=np.float32)
    weight = np.asarray(weight, dtype=np.float32)
    bias = np.asarray(bias, dtype=np.float32)

    in_maps = []
    for e in range(E):
        n = int(min(freq[e], TOK))
        x = np.zeros((TOK, IN), dtype=np.float32)
        x[:n] = input[starts[e]:starts[e] + n]
        xt = np.ascontiguousarray(x.T).astype(np_dt)          # [IN, TOK]
        # wt[o, p, c*128+j] = W[o*128+j, c*128+p]
        wswz = np.ascontiguousarray(
            weight[e].astype(np_dt).reshape(OT, 128, KC, 128)
            .transpose(0, 3, 2, 1)
        ).reshape(OT, 128, KC * 128)
        br = np.ascontiguousarray(bias[e].reshape(OT, 128).T)  # [128, OT]
        in_maps.append({"xt": xt, "wt": wswz, "bias": br})
    return in_maps, freq, starts


def _gather_out(results, freq, starts, n_tokens):
    out = np.zeros((n_tokens, OUT), dtype=np.float32)
    for e in range(E):
        n = int(min(freq[e], TOK))
        yt = np.asarray(results[e]["yt"])    # [OUT, TOK]
        out[starts[e]:starts[e] + n] = yt[:, :n].T
    return out


def kernel(input, weight, bias, expert_frequency, capacity=None, *,
           mode: str = MODE, trace: bool = False):
    """Full-input entry point: shards per expert across 8 cores, runs the
    Bass kernel, gathers the full [T, OUT] float32 output."""
    in_maps, freq, starts = _make_in_maps(
        input, weight, bias, expert_frequency, mode
    )
    nc = _get_nc(mode)
    res = run_bass_kernel_spmd(
        nc, in_maps, core_ids=list(range(E)), trace=trace
    )
    out = _gather_out(res.results, freq, starts, np.asarray(input).shape[0])
    if trace:
        return out, res
    return out


# revision 16
# speedup vs baseline: 1.0405x; 1.0405x over previous
"""Trainium2 kernel for nn_Experts (MoE grouped expert GEMM).

Problem: input [16384, 2048] f32, weight [8, 8192, 2048] f32, bias [8, 8192]
f32, expert_frequency [8] int32 (balanced: 2048 tokens/expert, pre-grouped),
capacity 2048.  Output [16384, 8192] f32 with out[t] = W_e x[t] + b_e.

Sharding: expert parallelism — core e computes expert e's GEMM
  Y_e = X_e @ W_e^T + b_e   (X_e [2048, 2048], W_e [8192, 2048])

Per-core kernel computes YT_e = W_e X_e^T + b_e  ([OUT, TOK], transposed
output; the host transposes back).  Matmul precision: single-pass bf16
(inputs rounded to bf16, fp32 PSUM accumulation) — carries ~2e-3 relative
error on this problem's randn data, comfortably under the 2e-2 gate, at
3x the throughput of the old split-precision bf16x3 scheme.

DMA-descriptor discipline (the previous bottleneck): every DMA moves
contiguous >=2KB lines per partition.  Weights are pre-swizzled on the
host to [OT, 128, KC*128] so one out-tile's weights are a single
128-descriptor (4KB/partition) transfer.  Output is written one out-tile
at a time ([128, TOK] f32, 8KB lines).

Engine layout (raw Bass, single-sem waits):
  SP   : input DMAs (w tile stream, x chunks, bias)
  PE   : 4096 matmuls ([128,128] stationary w x [128,512] moving x),
         k-outer: per (o,k) the stationary is reused for 4 consecutive
         t-slice matmuls; 8 PSUM banks, 4 per out-tile, ping-pong
  DVE  : PSUM -> SBUF eviction fused with per-partition bias add
  ACT  : output DMAs (one per out-tile)

Semaphore-correctness note: a DMA completion count of 16*n on a shared
ring proves only that SOME n DMAs finished, not WHICH — concurrent ring
DMAs complete out of order near stream start.  Tight per-chunk waits on
the x stream corrupted the first out-tiles nondeterministically; the PE
therefore waits for ALL x chunks (and w0+w1) before its first matmul.
All later waits carry >=10us of natural pipeline slack.  Measured:
~0.80 ms/core HW exec (PE streaming roofline ~0.87 ms), rel err 2.0e-3
vs the f32 reference (gate 2e-2).
"""

import numpy as np

import concourse.bass as bass
import concourse.mybir as mybir
from concourse.bass_utils import run_bass_kernel_spmd

# problem shape (per core)
E = 8
TOK = 2048      # tokens per expert (= capacity)
IN = 2048       # in features (contraction)
OUT = 8192      # out features
T_FULL = E * TOK

KC = IN // 128          # 16 contraction chunks
SLICE = 512             # moving-dim (token) slice = one PSUM bank
TS = TOK // SLICE       # 4 token slices
OT = OUT // 128         # 64 out tiles
NW = 4                  # w slot rotation
NP2 = 8                 # psum banks

F32 = mybir.dt.float32

_MODES = {
    # mode: mm dtype (single-pass, fp32 PSUM accumulate)
    "bf16": mybir.dt.bfloat16,
    "fp16": mybir.dt.float16,
}
MODE = "bf16"


def _enable_ldw_opt():
    """Flip walrus --enable-ldw-opt to true (elides identical consecutive
    LDWEIGHTS; the k-outer loop reuses each stationary 4x)."""
    import concourse.bass_utils as bu
    if getattr(bu.run_command, "_ldw_patched", False):
        return
    real_run = bu.run_command

    def run_hook(cmd, **kw):
        try:
            cmd = ["--enable-ldw-opt=true" if c == "--enable-ldw-opt=false" else c
                   for c in cmd]
        except Exception:
            pass
        return real_run(cmd, **kw)

    run_hook._ldw_patched = True
    bu.run_command = run_hook


def build(mode: str = MODE, reps: int = 1, bench: bool = False) -> bass.Bass:
    """reps: run the whole kernel body that many times back-to-back (for
    marginal-time benchmarking).  bench: make yt an internal DRAM scratch
    and expose only a tiny marker output, so per-call host<->device
    transfer is negligible during timing."""
    import os
    mm_dt = _MODES[mode]
    if os.environ.get("LDW_OPT", "1") != "0":
        # elide the 3 redundant LDWEIGHTS in each 4-matmul stationary-reuse
        # run: measured 330 ns/MM without (serialized 107ns reload per MM)
        _enable_ldw_opt()

    nc = bass.Bass(target_bir_lowering=False)
    xt = nc.dram_tensor("xt", [IN, TOK], mm_dt, kind="ExternalInput")
    wt = nc.dram_tensor("wt", [OT, 128, KC * 128], mm_dt, kind="ExternalInput")
    bias = nc.dram_tensor("bias", [128, OT], F32, kind="ExternalInput")
    if bench:
        yt = nc.dram_tensor("yt", [OUT, TOK], F32)  # internal scratch
        marker = nc.dram_tensor("marker", [128, OT], F32, kind="ExternalOutput")
    else:
        assert reps == 1
        yt = nc.dram_tensor("yt", [OUT, TOK], F32, kind="ExternalOutput")

    # [128, KC, TOK]: chunk c covers input rows c*128 .. c*128+127
    xt_r = xt[:, :].rearrange("(c p) t -> p c t", p=128)

    with (
        nc.sbuf_tensor("x_sb", [128, KC, TOK], mm_dt) as x_sb,
        nc.sbuf_tensor("w_sb", [128, NW, KC, 128], mm_dt) as w_sb,
        nc.sbuf_tensor("y_sb", [128, 2, TOK], F32) as y_sb,
        nc.sbuf_tensor("b_sb", [128, OT], F32) as b_sb,
        nc.psum_tensor("acc", [128, NP2, SLICE], F32) as acc,
        nc.semaphore("sem_x") as sem_x,
        nc.semaphore("sem_b") as sem_b,
        nc.semaphore("sem_w") as sem_w,
        nc.semaphore("sem_pe") as sem_pe,
        nc.semaphore("sem_dve") as sem_dve,
        nc.semaphore("sem_dout") as sem_dout,
        nc.Block() as block,
    ):
        WO = reps * OT
        x_done = [16 * (k + 1) for k in range(KC)]   # sem_x after chunk k

        @block.sync
        def _(sp):
            # w tile 0 first (PE's first dependency), then bias + x chunks
            # (PE consumes them k-ascending inside wo=0), then the w stream.
            sp.dma_start(w_sb[:, 0, :, :], wt[0]).then_inc(sem_w, 16)
            sp.dma_start(b_sb[:], bias[:]).then_inc(sem_b, 16)
            for k in range(KC):
                sp.dma_start(x_sb[:, k, :], xt_r[:, k, :]).then_inc(sem_x, 16)
            for wo in range(1, WO):
                o = wo % OT
                if wo >= NW:
                    # PE done reading slot wo-NW once sem_pe >= wo-NW+1
                    sp.wait_ge(sem_pe, wo - NW + 1)
                sp.dma_start(w_sb[:, wo % NW, :, :], wt[o]).then_inc(sem_w, 16)
            sp.wait_ge(sem_dout, 16 * WO)
            if bench:
                sp.dma_start(marker[:, :], b_sb[:]).then_inc(sem_x, 16)

        @block.tensor
        def _(pe):
            for wo in range(WO):
                pe.wait_ge(sem_w, 16 * (wo + 1) if wo else 32)
                if wo == 0:
                    # wait for ALL x chunks (and w0+w1): DMA completion
                    # counts don't identify WHICH dma finished, and ring
                    # completions reorder at stream start.  All later waits
                    # have >=10us of natural slack on top of the counts.
                    pe.wait_ge(sem_x, 16 * KC)
                if wo >= 2:
                    # banks (wo%2)*4.. freed once DVE evicted wo-2's tiles
                    pe.wait_ge(sem_dve, TS * (wo - 1))
                base = (wo % 2) * TS
                for k in range(KC):
                    for t in range(TS):
                        mm = pe.matmul(
                            acc[:, base + t, :],
                            w_sb[:, wo % NW, k, :],
                            x_sb[:, k, t * SLICE:(t + 1) * SLICE],
                            start=(k == 0),
                            stop=(k == KC - 1),
                        )
                mm.then_inc(sem_pe, 1)

        @block.vector
        def _(dve):
            for wo in range(WO):
                o = wo % OT
                dve.wait_ge(sem_pe, wo + 1)
                if wo == 0:
                    dve.wait_ge(sem_b, 16)
                if wo >= 2:
                    # y slot wo%2 free once wo-2's output DMA completed
                    dve.wait_ge(sem_dout, 16 * (wo - 1))
                base = (wo % 2) * TS
                for t in range(TS):
                    dve.tensor_scalar_add(
                        y_sb[:, wo % 2, t * SLICE:(t + 1) * SLICE],
                        acc[:, base + t, :],
                        b_sb[:, o:o + 1],
                    ).then_inc(sem_dve, 1)

        @block.scalar
        def _(act):
            for wo in range(WO):
                o = wo % OT
                act.wait_ge(sem_dve, TS * (wo + 1))
                act.dma_start(
                    yt[o * 128:(o + 1) * 128, :],
                    y_sb[:, wo % 2, :],
                ).then_inc(sem_dout, 16)

    return nc


def build_pe(mode: str = MODE, reps: int = 1, bench: bool = True) -> bass.Bass:
    """Bench-only probe: PE + weight-DMA loop with NO eviction/output path.
    Measures the pure matmul issue rate ceiling."""
    import os
    mm_dt = _MODES[mode]
    if os.environ.get("LDW_OPT", "1") != "0":
        # elide the 3 redundant LDWEIGHTS in each 4-matmul stationary-reuse
        # run: measured 330 ns/MM without (serialized 107ns reload per MM)
        _enable_ldw_opt()
    assert bench

    nc = bass.Bass(target_bir_lowering=False)
    xt = nc.dram_tensor("xt", [IN, TOK], mm_dt, kind="ExternalInput")
    wt = nc.dram_tensor("wt", [OT, 128, KC * 128], mm_dt, kind="ExternalInput")
    bias = nc.dram_tensor("bias", [128, OT], F32, kind="ExternalInput")
    marker = nc.dram_tensor("marker", [128, OT], F32, kind="ExternalOutput")

    xt_r = xt[:, :].rearrange("(c p) t -> p c t", p=128)

    with (
        nc.sbuf_tensor("x_sb", [128, KC, TOK], mm_dt) as x_sb,
        nc.sbuf_tensor("w_sb", [128, NW, KC, 128], mm_dt) as w_sb,
        nc.sbuf_tensor("b_sb", [128, OT], F32) as b_sb,
        nc.psum_tensor("acc", [128, NP2, SLICE], F32) as acc,
        nc.semaphore("sem_x") as sem_x,
        nc.semaphore("sem_b") as sem_b,
        nc.semaphore("sem_w") as sem_w,
        nc.semaphore("sem_pe") as sem_pe,
        nc.Block() as block,
    ):
        WO = reps * OT

        @block.sync
        def _(sp):
            sp.dma_start(w_sb[:, 0, :, :], wt[0]).then_inc(sem_w, 16)
            sp.dma_start(b_sb[:], bias[:]).then_inc(sem_b, 16)
            for k in range(KC):
                sp.dma_start(x_sb[:, k, :], xt_r[:, k, :]).then_inc(sem_x, 16)
            for wo in range(1, WO):
                o = wo % OT
                if wo >= NW:
                    sp.wait_ge(sem_pe, wo - NW + 1)
                sp.dma_start(w_sb[:, wo % NW, :, :], wt[o]).then_inc(sem_w, 16)
            sp.wait_ge(sem_pe, WO)
            sp.dma_start(marker[:, :], b_sb[:]).then_inc(sem_b, 16)

        @block.tensor
        def _(pe):
            for wo in range(WO):
                pe.wait_ge(sem_w, 16 * min(wo + 2, WO))
                if wo == 0:
                    pe.wait_ge(sem_x, 16 * KC)
                base = (wo % 2) * TS
                for k in range(KC):
                    for t in range(TS):
                        mm = pe.matmul(
                            acc[:, base + t, :],
                            w_sb[:, wo % NW, k, :],
                            x_sb[:, k, t * SLICE:(t + 1) * SLICE],
                            start=(k == 0),
                            stop=(k == KC - 1),
                        )
                mm.then_inc(sem_pe, 1)

    return nc


_nc_cache: dict = {}


def _get_nc(mode: str) -> bass.Bass:
    if mode not in _nc_cache:
        _nc_cache[mode] = build(mode)
    return _nc_cache[mode]


def _make_in_maps(input, weight, bias, expert_frequency, mode: str):
    mm_dt = _MODES[mode]
    np_dt = mybir.dt.np(mm_dt)

    freq = np.asarray(expert_frequency, dtype=np.int64)
    ends = np.cumsum(freq)
    starts = ends - freq

    input = np.asarray(input, dtype=np.float32)
    weight = np.asarray(weight, dtype=np.float32)
    bias = np.asarray(bias, dtype=np.float32)

    in_maps = []
    for e in range(E):
        n = int(min(freq[e], TOK))
        x = np.zeros((TOK, IN), dtype=np.float32)
        x[:n] = input[starts[e]:starts[e] + n]
        xt = np.ascontiguousarray(x.T).astype(np_dt)          # [IN, TOK]
        # wt[o, p, c*128+j] = W[o*128+j, c*128+p]
        wswz = np.ascontiguousarray(
            weight[e].astype(np_dt).reshape(OT, 128, KC, 128)
            .transpose(0, 3, 2, 1)
        ).reshape(OT, 128, KC * 128)
        br = np.ascontiguousarray(bias[e].reshape(OT, 128).T)  # [128, OT]
        in_maps.append({"xt": xt, "wt": wswz, "bias": br})
    return in_maps, freq, starts


def _gather_out(results, freq, starts, n_tokens):
    out = np.zeros((n_tokens, OUT), dtype=np.float32)
    for e in range(E):
        n = int(min(freq[e], TOK))
        yt = np.asarray(results[e]["yt"])    # [OUT, TOK]
        out[starts[e]:starts[e] + n] = yt[:, :n].T
    return out


def kernel(input, weight, bias, expert_frequency, capacity=None, *,
           mode: str = MODE, trace: bool = False):
    """Full-input entry point: shards per expert across 8 cores, runs the
    Bass kernel, gathers the full [T, OUT] float32 output."""
    in_maps, freq, starts = _make_in_maps(
        input, weight, bias, expert_frequency, mode
    )
    nc = _get_nc(mode)
    res = run_bass_kernel_spmd(
        nc, in_maps, core_ids=list(range(E)), trace=trace
    )
    out = _gather_out(res.results, freq, starts, np.asarray(input).shape[0])
    if trace:
        return out, res
    return out
